# revision 28
# baseline (speedup 1.0000x reference)
"""RGCN (relational GCN) layer on 8 Trainium2 NeuronCores.

out = relu(sum_r mean_{e: rel=r, dst=n} x[src_e] @ W_r + x @ root + bias)

Strategy: dst-node sharding (no collectives). Core c owns dst nodes
[c*6250, (c+1)*6250); every edge lives on its dst's owner core, so each core
computes its output slice independently and the host concatenates.

Device algorithm per core, per dst-tile j (128 dst nodes):
  - dma_gather x[src] rows (bf16) for all edges into per-call G tiles
    (x is split in two 25000-row halves so gather indices fit int16).
    Gather calls round-robin over 4 SWDGE queues - each queue's descriptors
    are generated by a different Q7 core pair, so independent calls (their
    own G tiles) run 4-way parallel.
  - The scatter matrices S (S[p, q] = w_p * (q == col_p), one [128, 128]
    bf16 tile per gather tile; col = dst_local, w = 1/count) are prebuilt
    on the host and streamed from HBM - no on-device S construction.
  - per relation window r (8 relations):
      bigPSUM[:, r*128:(r+1)*128] += G_t^T @ S_t over the window's tiles
      -> the per-(r, dst) *mean* in [feat, r*128+dst] layout.
  - self/root term: PE-transpose of the dst-tile's own x rows (streamed
    contiguously via xself) into window 8 of bigPSUM.
  - one ACT copy bigPSUM -> acc (bf16).
  - out^T[e, n] = sum_r W_r^T @ mean_r via 9 PSUM-accumulated matmuls
    (lhsT = wcat bf16 [d, e], rhs = acc slice [d, n]).
  - relu + bias in one ACT op (bias per-partition in transposed layout);
    DMA out. Host transposes each [e, n] tile back to [n, e].

All index preprocessing happens on the host; per-(window, half) tile counts
are maxed over the 8 cores so all cores run the same program (SPMD).
"""
import ml_dtypes
import numpy as np

import concourse.bass as bass
import concourse.mybir as mybir
import concourse.tile as tile
from concourse import bacc
from concourse.bass_utils import run_bass_kernel_spmd

N = 50000
E = 600000
D = 128
R = 8
P = 128
NCORES = 8
NC_NODES = N // NCORES          # 6250
NT = (NC_NODES + P - 1) // P    # 49
H = 25000                       # x half split (int16 index limit)
NR = R + 1                      # relations + self
MAXT = 8                        # <= 1024 idxs per dma_gather (16KB desc ring)
NQ = 4                          # SWDGE queues (one Q7 core pair each)

F32 = mybir.dt.float32
BF16 = mybir.dt.bfloat16
I16 = mybir.dt.int16
FP8 = mybir.dt.float8e4
BF = ml_dtypes.bfloat16
F8 = ml_dtypes.float8_e4m3


def _preprocess(edge_index, edge_type):
    """Core-invariant tile layout + per-core slot arrays (index data only).

    Slot layout per dst-tile j: [lo tiles (windows r=0..7) | hi tiles].
    """
    src = np.asarray(edge_index[0], dtype=np.int64)
    dst = np.asarray(edge_index[1], dtype=np.int64)
    et = np.asarray(edge_type, dtype=np.int64)

    counts = np.bincount(et * N + dst, minlength=R * N)

    core = dst // NC_NODES
    j = (dst - core * NC_NODES) // P
    half = (src >= H).astype(np.int64)

    key = ((core * NT + j) * R + et) * 2 + half
    cnt = np.bincount(key, minlength=NCORES * NT * R * 2).reshape(NCORES, NT, R, 2)
    tiles = -(-cnt // P)
    Tmax = tiles.max(axis=0)                   # [NT, R, 2]

    T_lo = Tmax[:, :, 0].copy()                # [NT, R]
    T_hi = Tmax[:, :, 1].copy()
    both0 = (T_lo + T_hi) == 0
    T_lo[both0] = 1

    Tlo_tot = T_lo.sum(axis=1)
    Thi_tot = T_hi.sum(axis=1)
    Tj = Tlo_tot + Thi_tot                     # gathered tiles per dst-tile

    lo_off = np.zeros((NT, R), dtype=np.int64)
    lo_off[:, 1:] = np.cumsum(T_lo, axis=1)[:, :-1]
    hi_off = np.zeros((NT, R), dtype=np.int64)
    hi_off[:, 1:] = np.cumsum(T_hi, axis=1)[:, :-1]
    hi_off += Tlo_tot[:, None]

    S_tiles = int(Tj.sum())

    jkey = core * NT + j
    order = np.lexsort((half, et, jkey))
    src_s, et_s, core_s, half_s = src[order], et[order], core[order], half[order]
    dst_s = dst[order]
    j_s = (dst_s - core_s * NC_NODES) // P
    col_s = (dst_s - core_s * NC_NODES) % P
    w_s = (1.0 / np.maximum(counts[et_s * N + dst_s], 1)).astype(np.float32)

    tile_base = np.zeros(NT, dtype=np.int64)
    tile_base[1:] = np.cumsum(Tj)[:-1]

    per_core = []
    for c in range(NCORES):
        m = core_s == c
        cs, cj, cr, ccol, cw, chalf = (a[m] for a in (src_s, j_s, et_s, col_s, w_s, half_s))
        gidx = np.zeros(S_tiles * P, dtype=np.int32)
        colv = np.full(S_tiles * P, -1.0, dtype=np.float32)  # -1 -> no S entry
        wv = np.zeros(S_tiles * P, dtype=np.float32)

        if len(cj):
            wkey = (cj * R + cr) * 2 + chalf
            changed = np.empty(len(wkey), dtype=bool)
            changed[0] = True
            changed[1:] = wkey[1:] != wkey[:-1]
            grp_start = np.maximum.accumulate(np.where(changed, np.arange(len(wkey)), 0))
            pos = np.arange(len(wkey)) - grp_start
            block = np.where(chalf == 0, lo_off[cj, cr], hi_off[cj, cr])
            slot = (tile_base[cj] + block + pos // P) * P + (pos % P)
            gidx[slot] = np.where(chalf == 0, cs, cs - H)
            colv[slot] = ccol
            wv[slot] = cw

        per_core.append({"gidx": gidx, "col": colv, "w": wv})

    layout = {
        "T_lo": T_lo, "T_hi": T_hi, "Tlo_tot": Tlo_tot, "Thi_tot": Thi_tot,
        "Tj": Tj, "lo_off": lo_off, "hi_off": hi_off, "S_tiles": S_tiles,
        "tile_base": tile_base,
    }
    return layout, per_core


def _wrap_idxs(flat):
    """dma_gather int16 index layout: idx i at [i%16, i//16], replicated x8."""
    a = np.asarray(flat, dtype=np.int16).reshape(-1, 16).T
    return np.tile(a, (8, 1))


def _call_ranges(tlo, thi):
    """Gather-call tile ranges per dst-tile: lo chunks then hi chunks."""
    out = []
    for t0 in range(0, tlo, MAXT):
        out.append((t0, min(t0 + MAXT, tlo)))
    for t0 in range(0, thi, MAXT):
        out.append((tlo + t0, tlo + min(t0 + MAXT, thi)))
    return out


def _mark_trailing_pad(gidx, wv, layout):
    """Set gather idx -1 on each gather call's trailing padding slots.

    Trailing negative idxs generate no DMA descriptors at all (the Q7 ucode
    trims them), saving descriptor-generation time. A call covers up to MAXT
    tiles of one (j, half) block, so only the final run of padding slots in
    each call qualifies; interior padding keeps idx 0 (its S row is zero).
    """
    Tj, Tlo_tot = layout["Tj"], layout["Tlo_tot"]
    tile_base = layout["tile_base"]
    g = gidx.copy()
    for j in range(NT):
        tg = int(Tj[j])
        tlo = int(Tlo_tot[j])
        thi = tg - tlo
        for c0, c1 in _call_ranges(tlo, thi):
            lo_s = (tile_base[j] + c0) * P
            hi_s = (tile_base[j] + c1) * P
            wspan = wv[lo_s:hi_s]
            nz = np.nonzero(wspan)[0]
            end = (nz[-1] + 1) if len(nz) else 0
            g[lo_s + end : hi_s] = -1
    return g


def _build_device_arrays(layout, per_core, x):
    """Per-core arrays: main [128, S_tiles*8] int16 idx, smat
    [128, S_tiles*128] bf16 (prebuilt scatter matrices), and xself
    [NT*128, 128] bf16 (the core's dst rows)."""
    S_tiles = layout["S_tiles"]
    x_bf = np.asarray(x, dtype=np.float32).astype(BF)
    out = []
    for c, meta in enumerate(per_core):
        gidx = _mark_trailing_pad(meta["gidx"], meta["w"], layout)
        colv, wv = meta["col"], meta["w"]

        # scatter matrices: smat[p, t*128 + col] = w for slot t*128+p
        smat = np.zeros((P, S_tiles * P), dtype=F8)
        slots = np.nonzero(colv >= 0)[0]
        part = slots % P
        colg = (slots // P) * P + colv[slots].astype(np.int64)
        smat[part, colg] = wv[slots].astype(F8)

        xs = np.zeros((NT * P, D), dtype=BF)
        n0 = c * NC_NODES
        xs[:NC_NODES] = x_bf[n0 : n0 + NC_NODES]
        out.append({
            "main": np.ascontiguousarray(_wrap_idxs(gidx), dtype=np.int16),
            "smat": np.ascontiguousarray(smat),
            "xself": xs,
        })
    return out


def _build_bass(layout, nt=NT, reps=1):
    T_lo, T_hi = layout["T_lo"], layout["T_hi"]
    Tj, Tlo_tot = layout["Tj"], layout["Tlo_tot"]
    lo_off, hi_off = layout["lo_off"], layout["hi_off"]
    S_tiles = layout["S_tiles"]

    nc = bacc.Bacc(None, target_bir_lowering=False, debug=False,
                   num_swdge_queues=NQ)

    xlo = nc.dram_tensor("xlo", [H, D], BF16, kind="ExternalInput")
    xhi = nc.dram_tensor("xhi", [H, D], BF16, kind="ExternalInput")
    xself = nc.dram_tensor("xself", [NT * P, D], BF16, kind="ExternalInput")
    main = nc.dram_tensor("main", [P, S_tiles * 8], I16, kind="ExternalInput")
    smat = nc.dram_tensor("smat", [P, S_tiles * P], FP8, kind="ExternalInput")
    wcat = nc.dram_tensor("wcat", [P, NR * P], BF16, kind="ExternalInput")
    biasc = nc.dram_tensor("biasc", [P, 1], F32, kind="ExternalInput")
    out_t = nc.dram_tensor("out_t", [nt, P, P], F32, kind="ExternalOutput")

    ident_np = np.eye(P, dtype=np.float32).astype(BF)
    ident_c = nc.inline_tensor(ident_np, name="ident_const")

    qctr = 0

    with tile.TileContext(nc) as tc:
        with (
            tc.tile_pool(name="const", bufs=1) as cpool,
            tc.tile_pool(name="g", bufs=30) as gpool,
            tc.tile_pool(name="gs", bufs=4) as gspool,
            tc.tile_pool(name="s", bufs=3) as spool,
            tc.tile_pool(name="acc", bufs=3) as apool,
            tc.tile_pool(name="o", bufs=3) as opool,
            tc.tile_pool(name="psw", bufs=2, space="PSUM") as pswin,
            tc.tile_pool(name="pso", bufs=2, space="PSUM") as psout,
        ):
            wcat_sb = cpool.tile([P, NR * P], BF16)
            nc.sync.dma_start(wcat_sb[:], wcat[:])
            bias_sb = cpool.tile([P, 1], F32)
            nc.sync.dma_start(bias_sb[:], biasc[:])
            ident_sb = cpool.tile([P, P], BF16)
            nc.sync.dma_start(ident_sb[:], ident_c[:])
            # whole gather-index array stays resident in SBUF (12.5KB per
            # partition): gather Q7 pairs on other SWDGE queues read idxs
            # asynchronously after instruction retire, so recycling idx
            # tiles through a pool races with in-flight gathers.
            main_sb = cpool.tile([P, S_tiles * 8], I16)
            nc.sync.dma_start(main_sb[:], main[:])



            for _rep in range(reps):
                for j in range(nt):
                    tg = int(Tj[j])
                    tlo = int(Tlo_tot[j])
                    thi = tg - tlo
                    base = int(layout["tile_base"][j])

                    S_sb = spool.tile([P, tg * P], FP8, tag="smat")
                    nc.sync.dma_start(S_sb[:], smat[:, base * P : (base + tg) * P])

                    # per-call G tiles: independent writes -> calls on
                    # different SWDGE queues overlap
                    calls = _call_ranges(tlo, thi)
                    gtiles = []
                    for (t0, t1) in calls:
                        Gc = gpool.tile([P, t1 - t0, P], BF16, tag="g")
                        src = xlo if t1 <= tlo else xhi
                        nc.gpsimd.dma_gather(
                            out_ap=Gc[:], in_ap=src[:],
                            idxs_ap=main_sb[:, (base + t0) * 8 : (base + t1) * 8],
                            num_idxs=(t1 - t0) * P, num_idxs_reg=(t1 - t0) * P,
                            elem_size=P,
                            queue_num=qctr % NQ,
                        )
                        qctr += 1
                        gtiles.append((t0, t1, Gc))
                    gself = gspool.tile([P, P], BF16, tag="gs")
                    nc.scalar.dma_start(gself[:], xself[j * P : (j + 1) * P, :])
                    # bound SWDGE completion out-of-orderness: periodically
                    # drain the pool engine's outstanding DMAs so G-tile
                    # buffer reuse (distance ~10 dst-tiles) always has a
                    # completed-DMA fence between writer and rewriter
                    if j % 6 == 5:
                        nc.gpsimd.drain()

                    def gblk(b):
                        for (t0, t1, Gc) in gtiles:
                            if t0 <= b < t1:
                                return Gc[:, b - t0, :]
                        raise AssertionError(b)

                    bps = pswin.tile([P, NR * P], F32, tag="psw")
                    for r in range(R):
                        blocks = [int(lo_off[j, r]) + t for t in range(int(T_lo[j, r]))] + \
                                 [int(hi_off[j, r]) + t for t in range(int(T_hi[j, r]))]
                        for k, b in enumerate(blocks):
                            nc.tensor.matmul(
                                bps[:, r * P : (r + 1) * P], lhsT=gblk(b),
                                rhs=S_sb[:, b * P : (b + 1) * P],
                                start=(k == 0), stop=(k == len(blocks) - 1),
                            )
                    # self/root window: mean_self = own x rows, transposed
                    # (regular matmul against identity: gself^T @ I)
                    nc.tensor.matmul(bps[:, R * P : NR * P], lhsT=gself[:],
                                     rhs=ident_sb[:], start=True, stop=True)

                    acc = apool.tile([P, NR * P], BF16, tag="acc")
                    nc.vector.tensor_copy(acc[:], bps[:])

                    ops = psout.tile([P, P], F32, tag="pso")
                    for r in range(NR):
                        nc.tensor.matmul(
                            ops[:], lhsT=wcat_sb[:, r * P : (r + 1) * P],
                            rhs=acc[:, r * P : (r + 1) * P],
                            start=(r == 0), stop=(r == NR - 1),
                        )
                    osb = opool.tile([P, P], F32, tag="o")
                    nc.vector.tensor_scalar(
                        out=osb[:], in0=ops[:],
                        scalar1=bias_sb[:, 0:1], scalar2=0.0,
                        op0=mybir.AluOpType.add, op1=mybir.AluOpType.max,
                    )
                    nc.sync.dma_start(out_t[j], osb[:])

    nc.compile()
    return nc


def _host_inputs(inputs):
    x = np.ascontiguousarray(np.asarray(inputs["x"]), dtype=np.float32)
    layout, per_core = _preprocess(np.asarray(inputs["edge_index"]),
                                   np.asarray(inputs["edge_type"]))
    dev = _build_device_arrays(layout, per_core, x)

    weight = np.asarray(inputs["weight"], np.float32)
    root = np.asarray(inputs["root"], np.float32)
    wcat = np.ascontiguousarray(
        np.concatenate([weight[r] for r in range(R)] + [root], axis=1)
    ).astype(BF)
    biasc = np.ascontiguousarray(np.asarray(inputs["bias"], np.float32)[:, None])
    x_bf = x.astype(BF)
    in_maps = [
        {"xlo": np.ascontiguousarray(x_bf[:H]), "xhi": np.ascontiguousarray(x_bf[H:]),
         "xself": dev[c]["xself"], "main": dev[c]["main"], "smat": dev[c]["smat"],
         "wcat": wcat, "biasc": biasc}
        for c in range(NCORES)
    ]
    return layout, in_maps


def kernel(x, edge_index, edge_type, weight, root, bias, _trace=False):
    inputs = {"x": x, "edge_index": edge_index, "edge_type": edge_type,
              "weight": weight, "root": root, "bias": bias}
    layout, in_maps = _host_inputs(inputs)
    nc = _build_bass(layout)
    res = run_bass_kernel_spmd(nc, in_maps, core_ids=list(range(NCORES)), trace=_trace)

    outs = []
    for c in range(NCORES):
        o = res.results[c]["out_t"].transpose(0, 2, 1).reshape(NT * P, D)
        outs.append(o[:NC_NODES])
    full = np.ascontiguousarray(np.concatenate(outs, axis=0), dtype=np.float32)
    if _trace:
        return full, res
    return full


# revision 36
# speedup vs baseline: 1.6922x; 1.6922x over previous
"""RGCN (relational GCN) layer on 8 Trainium2 NeuronCores.

out = relu(sum_r mean_{e: rel=r, dst=n} x[src_e] @ W_r + x @ root + bias)

Strategy: dst-node sharding (no collectives). Core c owns dst nodes
[c*6250, (c+1)*6250); every edge lives on its dst's owner core, so each core
computes its output slice independently and the host concatenates.

Device algorithm per core, per dst-tile j (128 dst nodes):
  - dma_gather x[src] rows (bf16) for all edges into per-call G tiles
    (x is split in two 25000-row halves so gather indices fit int16).
    Gather calls round-robin over 4 SWDGE queues - each queue's descriptors
    are generated by a different Q7 core pair, so independent calls (their
    own G tiles) run 4-way parallel.
  - The scatter matrices S (S[p, q] = w_p * (q == col_p), one [128, 128]
    bf16 tile per gather tile; col = dst_local, w = 1/count) are prebuilt
    on the host and streamed from HBM - no on-device S construction.
  - per relation window r (8 relations):
      bigPSUM[:, r*128:(r+1)*128] += G_t^T @ S_t over the window's tiles
      -> the per-(r, dst) *mean* in [feat, r*128+dst] layout.
  - self/root term: PE-transpose of the dst-tile's own x rows (streamed
    contiguously via xself) into window 8 of bigPSUM.
  - one ACT copy bigPSUM -> acc (bf16).
  - out^T[e, n] = sum_r W_r^T @ mean_r via 9 PSUM-accumulated matmuls
    (lhsT = wcat bf16 [d, e], rhs = acc slice [d, n]).
  - relu + bias in one ACT op (bias per-partition in transposed layout);
    DMA out. Host transposes each [e, n] tile back to [n, e].

All index preprocessing happens on the host; per-(window, half) tile counts
are maxed over the 8 cores so all cores run the same program (SPMD).
"""
import ml_dtypes
import numpy as np

import concourse.bass as bass
import concourse.mybir as mybir
import concourse.tile as tile
from concourse import bacc
from concourse.bass_utils import run_bass_kernel_spmd

N = 50000
E = 600000
D = 128
R = 8
P = 128
NCORES = 8
NC_NODES = N // NCORES          # 6250
NT = (NC_NODES + P - 1) // P    # 49
H = 25000                       # x half split (int16 index limit)
NR = R + 1                      # relations + self
MAXT = 8                        # <= 1024 idxs per dma_gather (16KB desc ring)
NQ = 4                          # SWDGE queues (one Q7 core pair each)

F32 = mybir.dt.float32
BF16 = mybir.dt.bfloat16
I16 = mybir.dt.int16
FP8 = mybir.dt.float8e4
BF = ml_dtypes.bfloat16
F8 = ml_dtypes.float8_e4m3


def _preprocess(edge_index, edge_type):
    """Core-invariant tile layout + per-core slot arrays (index data only).

    Slot layout per dst-tile j: [lo tiles (windows r=0..7) | hi tiles].
    """
    src = np.asarray(edge_index[0], dtype=np.int64)
    dst = np.asarray(edge_index[1], dtype=np.int64)
    et = np.asarray(edge_type, dtype=np.int64)

    counts = np.bincount(et * N + dst, minlength=R * N)

    core = dst // NC_NODES
    j = (dst - core * NC_NODES) // P
    half = (src >= H).astype(np.int64)

    key = ((core * NT + j) * R + et) * 2 + half
    cnt = np.bincount(key, minlength=NCORES * NT * R * 2).reshape(NCORES, NT, R, 2)
    tiles = -(-cnt // P)
    Tmax = tiles.max(axis=0)                   # [NT, R, 2]

    T_lo = Tmax[:, :, 0].copy()                # [NT, R]
    T_hi = Tmax[:, :, 1].copy()
    both0 = (T_lo + T_hi) == 0
    T_lo[both0] = 1

    Tlo_tot = T_lo.sum(axis=1)
    Thi_tot = T_hi.sum(axis=1)
    Tj = Tlo_tot + Thi_tot                     # gathered tiles per dst-tile

    lo_off = np.zeros((NT, R), dtype=np.int64)
    lo_off[:, 1:] = np.cumsum(T_lo, axis=1)[:, :-1]
    hi_off = np.zeros((NT, R), dtype=np.int64)
    hi_off[:, 1:] = np.cumsum(T_hi, axis=1)[:, :-1]
    hi_off += Tlo_tot[:, None]

    S_tiles = int(Tj.sum())

    jkey = core * NT + j
    order = np.lexsort((half, et, jkey))
    src_s, et_s, core_s, half_s = src[order], et[order], core[order], half[order]
    dst_s = dst[order]
    j_s = (dst_s - core_s * NC_NODES) // P
    col_s = (dst_s - core_s * NC_NODES) % P
    w_s = (1.0 / np.maximum(counts[et_s * N + dst_s], 1)).astype(np.float32)

    tile_base = np.zeros(NT, dtype=np.int64)
    tile_base[1:] = np.cumsum(Tj)[:-1]

    per_core = []
    for c in range(NCORES):
        m = core_s == c
        cs, cj, cr, ccol, cw, chalf = (a[m] for a in (src_s, j_s, et_s, col_s, w_s, half_s))
        gidx = np.zeros(S_tiles * P, dtype=np.int32)
        colv = np.full(S_tiles * P, -1.0, dtype=np.float32)  # -1 -> no S entry
        wv = np.zeros(S_tiles * P, dtype=np.float32)

        if len(cj):
            wkey = (cj * R + cr) * 2 + chalf
            changed = np.empty(len(wkey), dtype=bool)
            changed[0] = True
            changed[1:] = wkey[1:] != wkey[:-1]
            grp_start = np.maximum.accumulate(np.where(changed, np.arange(len(wkey)), 0))
            pos = np.arange(len(wkey)) - grp_start
            block = np.where(chalf == 0, lo_off[cj, cr], hi_off[cj, cr])
            slot = (tile_base[cj] + block + pos // P) * P + (pos % P)
            gidx[slot] = np.where(chalf == 0, cs, cs - H)
            colv[slot] = ccol
            wv[slot] = cw

        per_core.append({"gidx": gidx, "col": colv, "w": wv})

    layout = {
        "T_lo": T_lo, "T_hi": T_hi, "Tlo_tot": Tlo_tot, "Thi_tot": Thi_tot,
        "Tj": Tj, "lo_off": lo_off, "hi_off": hi_off, "S_tiles": S_tiles,
        "tile_base": tile_base,
    }
    return layout, per_core


def _wrap_idxs(flat):
    """dma_gather int16 index layout: idx i at [i%16, i//16], replicated x8."""
    a = np.asarray(flat, dtype=np.int16).reshape(-1, 16).T
    return np.tile(a, (8, 1))


def _call_ranges(tlo, thi):
    """Gather-call tile ranges per dst-tile: lo chunks then hi chunks."""
    out = []
    for t0 in range(0, tlo, MAXT):
        out.append((t0, min(t0 + MAXT, tlo)))
    for t0 in range(0, thi, MAXT):
        out.append((tlo + t0, tlo + min(t0 + MAXT, thi)))
    return out


def _call_list(layout):
    """Flat list of (j, t0, t1) gather calls in emission order."""
    Tj, Tlo_tot = layout["Tj"], layout["Tlo_tot"]
    calls = []
    for j in range(NT):
        tg = int(Tj[j])
        tlo = int(Tlo_tot[j])
        thi = tg - tlo
        for (t0, t1) in _call_ranges(tlo, thi):
            calls.append((j, t0, t1))
    return calls


def _build_device_arrays(layout, per_core, x):
    """Per-core arrays: main [128, ncalls*64] int16 idx (one fixed-size
    1024-idx block per gather call, -1 padded - trailing negatives generate
    no descriptors), smat [128, S_tiles*128] bf16 (prebuilt scatter
    matrices), and xself [NT*128, 128] bf16 (the core's dst rows)."""
    S_tiles = layout["S_tiles"]
    tile_base = layout["tile_base"]
    calls = _call_list(layout)
    x_bf = np.asarray(x, dtype=np.float32).astype(BF)
    out = []
    for c, meta in enumerate(per_core):
        gidx, colv, wv = meta["gidx"], meta["col"], meta["w"]

        # scatter matrices: smat[p, t*128 + col] = w for slot t*128+p
        smat = np.zeros((P, S_tiles * P), dtype=BF)
        slots = np.nonzero(colv >= 0)[0]
        part = slots % P
        colg = (slots // P) * P + colv[slots].astype(np.int64)
        smat[part, colg] = wv[slots].astype(BF)

        # per-call fixed-size idx blocks: real slots up to the last real
        # edge, then -1 (the Q7 ucode trims trailing negatives: zero cost)
        blocks = []
        for (j, t0, t1) in calls:
            lo_s = (tile_base[j] + t0) * P
            hi_s = (tile_base[j] + t1) * P
            span = gidx[lo_s:hi_s]
            nz = np.nonzero(wv[lo_s:hi_s])[0]
            end = (nz[-1] + 1) if len(nz) else 0
            blk = np.full(MAXT * P, -1, dtype=np.int32)
            blk[:end] = span[:end]
            blocks.append(blk)
        main = _wrap_idxs(np.concatenate(blocks))

        xs = np.zeros((NT * P, D), dtype=BF)
        n0 = c * NC_NODES
        xs[:NC_NODES] = x_bf[n0 : n0 + NC_NODES]
        out.append({
            "main": np.ascontiguousarray(main, dtype=np.int16),
            "smat": np.ascontiguousarray(smat),
            "xself": xs,
        })
    return out


def _build_bass(layout, nt=NT, reps=1):
    T_lo, T_hi = layout["T_lo"], layout["T_hi"]
    Tj, Tlo_tot = layout["Tj"], layout["Tlo_tot"]
    lo_off, hi_off = layout["lo_off"], layout["hi_off"]
    S_tiles = layout["S_tiles"]

    nc = bacc.Bacc(None, target_bir_lowering=False, debug=False,
                   num_swdge_queues=NQ)

    all_calls = _call_list(layout)
    ncalls = len(all_calls)

    xlo = nc.dram_tensor("xlo", [H, D], BF16, kind="ExternalInput")
    xhi = nc.dram_tensor("xhi", [H, D], BF16, kind="ExternalInput")
    xself = nc.dram_tensor("xself", [NT * P, D], BF16, kind="ExternalInput")
    main = nc.dram_tensor("main", [P, ncalls * MAXT * 8], I16, kind="ExternalInput")
    smat = nc.dram_tensor("smat", [P, S_tiles * P], BF16, kind="ExternalInput")
    wcat = nc.dram_tensor("wcat", [P, NR * P], BF16, kind="ExternalInput")
    biasc = nc.dram_tensor("biasc", [P, 1], F32, kind="ExternalInput")
    out_t = nc.dram_tensor("out_t", [nt, P, P], F32, kind="ExternalOutput")

    ident_np = np.eye(P, dtype=np.float32).astype(BF)
    ident_c = nc.inline_tensor(ident_np, name="ident_const")

    qctr = 0

    with tile.TileContext(nc) as tc:
        with (
            tc.tile_pool(name="const", bufs=1) as cpool,
            tc.tile_pool(name="g", bufs=24) as gpool,
            tc.tile_pool(name="gs", bufs=4) as gspool,
            tc.tile_pool(name="s", bufs=3) as spool,
            tc.tile_pool(name="acc", bufs=3) as apool,
            tc.tile_pool(name="o", bufs=3) as opool,
            tc.tile_pool(name="psw", bufs=2, space="PSUM") as pswin,
            tc.tile_pool(name="pso", bufs=2, space="PSUM") as psout,
        ):
            wcat_sb = cpool.tile([P, NR * P], BF16)
            nc.sync.dma_start(wcat_sb[:], wcat[:])
            bias_sb = cpool.tile([P, 1], F32)
            nc.sync.dma_start(bias_sb[:], biasc[:])
            ident_sb = cpool.tile([P, P], BF16)
            nc.sync.dma_start(ident_sb[:], ident_c[:])
            # whole gather-index array stays resident in SBUF (~13KB per
            # partition): gather Q7 pairs on other SWDGE queues read idxs
            # asynchronously after instruction retire, so recycling idx
            # tiles through a pool races with in-flight gathers.
            main_sb = cpool.tile([P, ncalls * MAXT * 8], I16)
            nc.sync.dma_start(main_sb[:], main[:])
            # every call uses the same 1024-idx size (trailing -1 idxs are
            # trimmed by the Q7 ucode at zero cost), so one register feeds
            # all gathers - a per-call MOVE would be one more broadcast
            # instruction between gathers, clogging the Q7 dispatch FIFO.
            nidx_reg = nc.gpsimd.to_reg(MAXT * P)



            for _rep in range(reps):
                for j in range(nt):
                    tg = int(Tj[j])
                    tlo = int(Tlo_tot[j])
                    thi = tg - tlo
                    base = int(layout["tile_base"][j])

                    S_sb = spool.tile([P, tg * P], BF16, tag="smat")
                    nc.sync.dma_start(S_sb[:], smat[:, base * P : (base + tg) * P])

                    # per-call G tiles: independent writes -> calls on
                    # different SWDGE queues overlap
                    calls = _call_ranges(tlo, thi)
                    gtiles = []
                    for (t0, t1) in calls:
                        Gc = gpool.tile([P, MAXT, P], BF16, tag="g")
                        src = xlo if t1 <= tlo else xhi
                        nt_call = t1 - t0
                        nc.gpsimd.dma_gather(
                            out_ap=Gc[:, :nt_call, :], in_ap=src[:],
                            idxs_ap=main_sb[:, qctr * MAXT * 8 : qctr * MAXT * 8 + nt_call * 8],
                            num_idxs=nt_call * P, num_idxs_reg=nt_call * P,
                            elem_size=P,
                            queue_num=qctr % NQ,
                        )
                        qctr += 1
                        gtiles.append((t0, t1, Gc))
                    gself = gspool.tile([P, P], BF16, tag="gs")
                    nc.scalar.dma_start(gself[:], xself[j * P : (j + 1) * P, :])
                    # bound SWDGE completion out-of-orderness: periodically
                    # drain the pool engine's outstanding DMAs so G-tile
                    # buffer reuse (distance ~10 dst-tiles) always has a
                    # completed-DMA fence between writer and rewriter
                    if j % 6 == 5:
                        nc.gpsimd.drain()

                    def gblk(b):
                        for (t0, t1, Gc) in gtiles:
                            if t0 <= b < t1:
                                return Gc[:, b - t0, :]
                        raise AssertionError(b)

                    bps = pswin.tile([P, NR * P], F32, tag="psw")
                    for r in range(R):
                        blocks = [int(lo_off[j, r]) + t for t in range(int(T_lo[j, r]))] + \
                                 [int(hi_off[j, r]) + t for t in range(int(T_hi[j, r]))]
                        for k, b in enumerate(blocks):
                            nc.tensor.matmul(
                                bps[:, r * P : (r + 1) * P], lhsT=gblk(b),
                                rhs=S_sb[:, b * P : (b + 1) * P],
                                start=(k == 0), stop=(k == len(blocks) - 1),
                            )
                    # self/root window: mean_self = own x rows, transposed
                    # (regular matmul against identity: gself^T @ I)
                    nc.tensor.matmul(bps[:, R * P : NR * P], lhsT=gself[:],
                                     rhs=ident_sb[:], start=True, stop=True)

                    acc = apool.tile([P, NR * P], BF16, tag="acc")
                    nc.vector.tensor_copy(acc[:], bps[:])

                    ops = psout.tile([P, P], F32, tag="pso")
                    for r in range(NR):
                        nc.tensor.matmul(
                            ops[:], lhsT=wcat_sb[:, r * P : (r + 1) * P],
                            rhs=acc[:, r * P : (r + 1) * P],
                            start=(r == 0), stop=(r == NR - 1),
                        )
                    osb = opool.tile([P, P], F32, tag="o")
                    nc.vector.tensor_scalar(
                        out=osb[:], in0=ops[:],
                        scalar1=bias_sb[:, 0:1], scalar2=0.0,
                        op0=mybir.AluOpType.add, op1=mybir.AluOpType.max,
                    )
                    nc.sync.dma_start(out_t[j], osb[:])

    nc.compile()
    return nc


def _host_inputs(inputs):
    x = np.ascontiguousarray(np.asarray(inputs["x"]), dtype=np.float32)
    layout, per_core = _preprocess(np.asarray(inputs["edge_index"]),
                                   np.asarray(inputs["edge_type"]))
    dev = _build_device_arrays(layout, per_core, x)

    weight = np.asarray(inputs["weight"], np.float32)
    root = np.asarray(inputs["root"], np.float32)
    wcat = np.ascontiguousarray(
        np.concatenate([weight[r] for r in range(R)] + [root], axis=1)
    ).astype(BF)
    biasc = np.ascontiguousarray(np.asarray(inputs["bias"], np.float32)[:, None])
    x_bf = x.astype(BF)
    in_maps = [
        {"xlo": np.ascontiguousarray(x_bf[:H]), "xhi": np.ascontiguousarray(x_bf[H:]),
         "xself": dev[c]["xself"], "main": dev[c]["main"], "smat": dev[c]["smat"],
         "wcat": wcat, "biasc": biasc}
        for c in range(NCORES)
    ]
    return layout, in_maps


def kernel(x, edge_index, edge_type, weight, root, bias, _trace=False):
    inputs = {"x": x, "edge_index": edge_index, "edge_type": edge_type,
              "weight": weight, "root": root, "bias": bias}
    layout, in_maps = _host_inputs(inputs)
    nc = _build_bass(layout)
    res = run_bass_kernel_spmd(nc, in_maps, core_ids=list(range(NCORES)), trace=_trace)

    outs = []
    for c in range(NCORES):
        o = res.results[c]["out_t"].transpose(0, 2, 1).reshape(NT * P, D)
        outs.append(o[:NC_NODES])
    full = np.ascontiguousarray(np.concatenate(outs, axis=0), dtype=np.float32)
    if _trace:
        return full, res
    return full


# revision 39
# speedup vs baseline: 1.7585x; 1.0391x over previous
"""RGCN (relational GCN) layer on 8 Trainium2 NeuronCores.

out = relu(sum_r mean_{e: rel=r, dst=n} x[src_e] @ W_r + x @ root + bias)

Strategy: dst-node sharding (no collectives). Core c owns dst nodes
[c*6250, (c+1)*6250); every edge lives on its dst's owner core, so each core
computes its output slice independently and the host concatenates.

Device algorithm per core, per dst-tile j (128 dst nodes):
  - dma_gather x[src] rows (bf16) for all edges into per-call G tiles
    (x is split in two 25000-row halves so gather indices fit int16).
    Gather calls round-robin over 4 SWDGE queues - each queue's descriptors
    are generated by a different Q7 core pair, so independent calls (their
    own G tiles) run 4-way parallel.
  - The scatter matrices S (S[p, q] = w_p * (q == col_p), one [128, 128]
    bf16 tile per gather tile; col = dst_local, w = 1/count) are prebuilt
    on the host and streamed from HBM - no on-device S construction.
  - per relation window r (8 relations):
      bigPSUM[:, r*128:(r+1)*128] += G_t^T @ S_t over the window's tiles
      -> the per-(r, dst) *mean* in [feat, r*128+dst] layout.
  - self/root term: PE-transpose of the dst-tile's own x rows (streamed
    contiguously via xself) into window 8 of bigPSUM.
  - one ACT copy bigPSUM -> acc (bf16).
  - out^T[e, n] = sum_r W_r^T @ mean_r via 9 PSUM-accumulated matmuls
    (lhsT = wcat bf16 [d, e], rhs = acc slice [d, n]).
  - relu + bias in one ACT op (bias per-partition in transposed layout);
    DMA out. Host transposes each [e, n] tile back to [n, e].

All index preprocessing happens on the host; per-(window, half) tile counts
are maxed over the 8 cores so all cores run the same program (SPMD).
"""
import ml_dtypes
import numpy as np

import concourse.bass as bass
import concourse.mybir as mybir
import concourse.tile as tile
from concourse import bacc
from concourse.bass_utils import run_bass_kernel_spmd

N = 50000
E = 600000
D = 128
R = 8
P = 128
NCORES = 8
NC_NODES = N // NCORES          # 6250
NT = (NC_NODES + P - 1) // P    # 49
H = 25000                       # x half split (int16 index limit)
NR = R + 1                      # relations + self
MAXT = 8                        # <= 1024 idxs per dma_gather (16KB desc ring)
NQ = 4                          # SWDGE queues (one Q7 core pair each)

F32 = mybir.dt.float32
BF16 = mybir.dt.bfloat16
I16 = mybir.dt.int16
FP8 = mybir.dt.float8e4
BF = ml_dtypes.bfloat16
F8 = ml_dtypes.float8_e4m3


def _preprocess(edge_index, edge_type):
    """Core-invariant tile layout + per-core slot arrays (index data only).

    Slot layout per dst-tile j: [lo tiles (windows r=0..7) | hi tiles].
    """
    src = np.asarray(edge_index[0], dtype=np.int64)
    dst = np.asarray(edge_index[1], dtype=np.int64)
    et = np.asarray(edge_type, dtype=np.int64)

    counts = np.bincount(et * N + dst, minlength=R * N)

    core = dst // NC_NODES
    j = (dst - core * NC_NODES) // P
    half = (src >= H).astype(np.int64)

    key = ((core * NT + j) * R + et) * 2 + half
    cnt = np.bincount(key, minlength=NCORES * NT * R * 2).reshape(NCORES, NT, R, 2)
    tiles = -(-cnt // P)
    Tmax = tiles.max(axis=0)                   # [NT, R, 2]

    T_lo = Tmax[:, :, 0].copy()                # [NT, R]
    T_hi = Tmax[:, :, 1].copy()
    both0 = (T_lo + T_hi) == 0
    T_lo[both0] = 1

    Tlo_tot = T_lo.sum(axis=1)
    Thi_tot = T_hi.sum(axis=1)
    Tj = Tlo_tot + Thi_tot                     # gathered tiles per dst-tile

    lo_off = np.zeros((NT, R), dtype=np.int64)
    lo_off[:, 1:] = np.cumsum(T_lo, axis=1)[:, :-1]
    hi_off = np.zeros((NT, R), dtype=np.int64)
    hi_off[:, 1:] = np.cumsum(T_hi, axis=1)[:, :-1]
    hi_off += Tlo_tot[:, None]

    S_tiles = int(Tj.sum())

    jkey = core * NT + j
    order = np.lexsort((half, et, jkey))
    src_s, et_s, core_s, half_s = src[order], et[order], core[order], half[order]
    dst_s = dst[order]
    j_s = (dst_s - core_s * NC_NODES) // P
    col_s = (dst_s - core_s * NC_NODES) % P
    w_s = (1.0 / np.maximum(counts[et_s * N + dst_s], 1)).astype(np.float32)

    tile_base = np.zeros(NT, dtype=np.int64)
    tile_base[1:] = np.cumsum(Tj)[:-1]

    per_core = []
    for c in range(NCORES):
        m = core_s == c
        cs, cj, cr, ccol, cw, chalf = (a[m] for a in (src_s, j_s, et_s, col_s, w_s, half_s))
        gidx = np.zeros(S_tiles * P, dtype=np.int32)
        colv = np.full(S_tiles * P, -1.0, dtype=np.float32)  # -1 -> no S entry
        wv = np.zeros(S_tiles * P, dtype=np.float32)

        if len(cj):
            wkey = (cj * R + cr) * 2 + chalf
            changed = np.empty(len(wkey), dtype=bool)
            changed[0] = True
            changed[1:] = wkey[1:] != wkey[:-1]
            grp_start = np.maximum.accumulate(np.where(changed, np.arange(len(wkey)), 0))
            pos = np.arange(len(wkey)) - grp_start
            block = np.where(chalf == 0, lo_off[cj, cr], hi_off[cj, cr])
            slot = (tile_base[cj] + block + pos // P) * P + (pos % P)
            gidx[slot] = np.where(chalf == 0, cs, cs - H)
            colv[slot] = ccol
            wv[slot] = cw

        per_core.append({"gidx": gidx, "col": colv, "w": wv})

    layout = {
        "T_lo": T_lo, "T_hi": T_hi, "Tlo_tot": Tlo_tot, "Thi_tot": Thi_tot,
        "Tj": Tj, "lo_off": lo_off, "hi_off": hi_off, "S_tiles": S_tiles,
        "tile_base": tile_base,
    }
    return layout, per_core


def _wrap_idxs(flat):
    """dma_gather int16 index layout: idx i at [i%16, i//16], replicated x8."""
    a = np.asarray(flat, dtype=np.int16).reshape(-1, 16).T
    return np.tile(a, (8, 1))


def _call_ranges(tlo, thi):
    """Gather-call tile ranges per dst-tile: lo chunks then hi chunks."""
    out = []
    for t0 in range(0, tlo, MAXT):
        out.append((t0, min(t0 + MAXT, tlo)))
    for t0 in range(0, thi, MAXT):
        out.append((tlo + t0, tlo + min(t0 + MAXT, thi)))
    return out


def _call_list(layout):
    """Flat list of (j, t0, t1) gather calls in emission order."""
    Tj, Tlo_tot = layout["Tj"], layout["Tlo_tot"]
    calls = []
    for j in range(NT):
        tg = int(Tj[j])
        tlo = int(Tlo_tot[j])
        thi = tg - tlo
        for (t0, t1) in _call_ranges(tlo, thi):
            calls.append((j, t0, t1))
    return calls


def _build_device_arrays(layout, per_core, x):
    """Per-core arrays: main [128, ncalls*64] int16 idx (one fixed-size
    1024-idx block per gather call, -1 padded - trailing negatives generate
    no descriptors), smat [128, S_tiles*128] bf16 (prebuilt scatter
    matrices), and xself [NT*128, 128] bf16 (the core's dst rows)."""
    S_tiles = layout["S_tiles"]
    tile_base = layout["tile_base"]
    calls = _call_list(layout)
    x_bf = np.asarray(x, dtype=np.float32).astype(BF)
    out = []
    for c, meta in enumerate(per_core):
        gidx, colv, wv = meta["gidx"], meta["col"], meta["w"]

        # scatter matrices: smat[p, t*128 + col] = w for slot t*128+p
        smat = np.zeros((P, S_tiles * P), dtype=BF)
        slots = np.nonzero(colv >= 0)[0]
        part = slots % P
        colg = (slots // P) * P + colv[slots].astype(np.int64)
        smat[part, colg] = wv[slots].astype(BF)

        # per-call fixed-size idx blocks: real slots up to the last real
        # edge, then -1 (the Q7 ucode trims trailing negatives: zero cost)
        blocks = []
        for (j, t0, t1) in calls:
            lo_s = (tile_base[j] + t0) * P
            hi_s = (tile_base[j] + t1) * P
            span = gidx[lo_s:hi_s]
            nz = np.nonzero(wv[lo_s:hi_s])[0]
            end = (nz[-1] + 1) if len(nz) else 0
            blk = np.full(MAXT * P, -1, dtype=np.int32)
            blk[:end] = span[:end]
            blocks.append(blk)
        main = _wrap_idxs(np.concatenate(blocks))

        xs = np.zeros((NT * P, D), dtype=BF)
        n0 = c * NC_NODES
        xs[:NC_NODES] = x_bf[n0 : n0 + NC_NODES]
        out.append({
            "main": np.ascontiguousarray(main, dtype=np.int16),
            "smat": np.ascontiguousarray(smat),
            "xself": xs,
        })
    return out


def _build_bass(layout, nt=NT, reps=1):
    T_lo, T_hi = layout["T_lo"], layout["T_hi"]
    Tj, Tlo_tot = layout["Tj"], layout["Tlo_tot"]
    lo_off, hi_off = layout["lo_off"], layout["hi_off"]
    S_tiles = layout["S_tiles"]

    nc = bacc.Bacc(None, target_bir_lowering=False, debug=False,
                   num_swdge_queues=NQ)

    all_calls = _call_list(layout)
    ncalls = len(all_calls)

    xlo = nc.dram_tensor("xlo", [H, D], BF16, kind="ExternalInput")
    xhi = nc.dram_tensor("xhi", [H, D], BF16, kind="ExternalInput")
    xself = nc.dram_tensor("xself", [NT * P, D], BF16, kind="ExternalInput")
    main = nc.dram_tensor("main", [P, ncalls * MAXT * 8], I16, kind="ExternalInput")
    smat = nc.dram_tensor("smat", [P, S_tiles * P], BF16, kind="ExternalInput")
    wcat = nc.dram_tensor("wcat", [P, NR * P], BF16, kind="ExternalInput")
    biasc = nc.dram_tensor("biasc", [P, 1], F32, kind="ExternalInput")
    out_t = nc.dram_tensor("out_t", [nt, P, P], F32, kind="ExternalOutput")

    ident_np = np.eye(P, dtype=np.float32).astype(BF)
    ident_c = nc.inline_tensor(ident_np, name="ident_const")

    qctr = 0

    with tile.TileContext(nc) as tc:
        with (
            tc.tile_pool(name="const", bufs=1) as cpool,
            tc.tile_pool(name="g", bufs=24) as gpool,
            tc.tile_pool(name="gs", bufs=4) as gspool,
            tc.tile_pool(name="s", bufs=3) as spool,
            tc.tile_pool(name="acc", bufs=3) as apool,
            tc.tile_pool(name="o", bufs=3) as opool,
            tc.tile_pool(name="psw", bufs=2, space="PSUM") as pswin,
            tc.tile_pool(name="pso", bufs=2, space="PSUM") as psout,
        ):
            wcat_sb = cpool.tile([P, NR * P], BF16)
            nc.sync.dma_start(wcat_sb[:], wcat[:])
            bias_sb = cpool.tile([P, 1], F32)
            nc.sync.dma_start(bias_sb[:], biasc[:])
            ident_sb = cpool.tile([P, P], BF16)
            nc.sync.dma_start(ident_sb[:], ident_c[:])
            # whole gather-index array stays resident in SBUF (~13KB per
            # partition): gather Q7 pairs on other SWDGE queues read idxs
            # asynchronously after instruction retire, so recycling idx
            # tiles through a pool races with in-flight gathers.
            main_sb = cpool.tile([P, ncalls * MAXT * 8], I16)
            nc.sync.dma_start(main_sb[:], main[:])
            # every call uses the same 1024-idx size (trailing -1 idxs are
            # trimmed by the Q7 ucode at zero cost), so one register feeds
            # all gathers - a per-call MOVE would be one more broadcast
            # instruction between gathers, clogging the Q7 dispatch FIFO.
            nidx_reg = nc.gpsimd.to_reg(MAXT * P)



            for _rep in range(reps):
                for j in range(nt):
                    tg = int(Tj[j])
                    tlo = int(Tlo_tot[j])
                    thi = tg - tlo
                    base = int(layout["tile_base"][j])

                    S_sb = spool.tile([P, tg * P], BF16, tag="smat")
                    nc.sync.dma_start(S_sb[:], smat[:, base * P : (base + tg) * P])

                    # per-call G tiles: independent writes -> calls on
                    # different SWDGE queues overlap
                    calls = _call_ranges(tlo, thi)
                    gtiles = []
                    for (t0, t1) in calls:
                        Gc = gpool.tile([P, MAXT, P], BF16, tag="g")
                        src = xlo if t1 <= tlo else xhi
                        nt_call = t1 - t0
                        nc.gpsimd.dma_gather(
                            out_ap=Gc[:, :nt_call, :], in_ap=src[:],
                            idxs_ap=main_sb[:, qctr * MAXT * 8 : qctr * MAXT * 8 + nt_call * 8],
                            num_idxs=nt_call * P, num_idxs_reg=nt_call * P,
                            elem_size=P,
                            queue_num=qctr % NQ,
                        )
                        qctr += 1
                        gtiles.append((t0, t1, Gc))
                    gself = gspool.tile([P, P], BF16, tag="gs")
                    nc.scalar.dma_start(gself[:], xself[j * P : (j + 1) * P, :])
                    # bound SWDGE completion out-of-orderness: periodically
                    # drain the pool engine's outstanding DMAs so G-tile
                    # buffer reuse (distance ~10 dst-tiles) always has a
                    # completed-DMA fence between writer and rewriter
                    if j % 6 == 5:
                        nc.gpsimd.drain()

                    def gblk(b):
                        for (t0, t1, Gc) in gtiles:
                            if t0 <= b < t1:
                                return Gc[:, b - t0, :]
                        raise AssertionError(b)

                    bps = pswin.tile([P, NR * P], F32, tag="psw")
                    for r in range(R):
                        blocks = [int(lo_off[j, r]) + t for t in range(int(T_lo[j, r]))] + \
                                 [int(hi_off[j, r]) + t for t in range(int(T_hi[j, r]))]
                        for k, b in enumerate(blocks):
                            nc.tensor.matmul(
                                bps[:, r * P : (r + 1) * P], lhsT=gblk(b),
                                rhs=S_sb[:, b * P : (b + 1) * P],
                                start=(k == 0), stop=(k == len(blocks) - 1),
                            )
                    # self/root window: mean_self = own x rows, transposed
                    # (regular matmul against identity: gself^T @ I)
                    nc.tensor.matmul(bps[:, R * P : NR * P], lhsT=gself[:],
                                     rhs=ident_sb[:], start=True, stop=True)

                    acc = apool.tile([P, NR * P], BF16, tag="acc")
                    nc.vector.tensor_copy(acc[:], bps[:])

                    ops = psout.tile([P, P], F32, tag="pso")
                    for r in range(NR):
                        nc.tensor.matmul(
                            ops[:], lhsT=wcat_sb[:, r * P : (r + 1) * P],
                            rhs=acc[:, r * P : (r + 1) * P],
                            start=(r == 0), stop=(r == NR - 1),
                        )
                    osb = opool.tile([P, P], F32, tag="o")
                    nc.vector.tensor_scalar(
                        out=osb[:], in0=ops[:],
                        scalar1=bias_sb[:, 0:1], scalar2=0.0,
                        op0=mybir.AluOpType.add, op1=mybir.AluOpType.max,
                    )
                    nc.sync.dma_start(out_t[j], osb[:])

    nc.compile()
    return nc


def _host_inputs(inputs):
    x = np.ascontiguousarray(np.asarray(inputs["x"]), dtype=np.float32)
    layout, per_core = _preprocess(np.asarray(inputs["edge_index"]),
                                   np.asarray(inputs["edge_type"]))
    dev = _build_device_arrays(layout, per_core, x)

    weight = np.asarray(inputs["weight"], np.float32)
    root = np.asarray(inputs["root"], np.float32)
    wcat = np.ascontiguousarray(
        np.concatenate([weight[r] for r in range(R)] + [root], axis=1)
    ).astype(BF)
    biasc = np.ascontiguousarray(np.asarray(inputs["bias"], np.float32)[:, None])
    x_bf = x.astype(BF)
    in_maps = [
        {"xlo": np.ascontiguousarray(x_bf[:H]), "xhi": np.ascontiguousarray(x_bf[H:]),
         "xself": dev[c]["xself"], "main": dev[c]["main"], "smat": dev[c]["smat"],
         "wcat": wcat, "biasc": biasc}
        for c in range(NCORES)
    ]
    return layout, in_maps


def kernel(x, edge_index, edge_type, weight, root, bias, _trace=False):
    inputs = {"x": x, "edge_index": edge_index, "edge_type": edge_type,
              "weight": weight, "root": root, "bias": bias}
    layout, in_maps = _host_inputs(inputs)
    nc = _build_bass(layout)
    res = run_bass_kernel_spmd(nc, in_maps, core_ids=list(range(NCORES)), trace=_trace)

    outs = []
    for c in range(NCORES):
        o = res.results[c]["out_t"].transpose(0, 2, 1).reshape(NT * P, D)
        outs.append(o[:NC_NODES])
    full = np.ascontiguousarray(np.concatenate(outs, axis=0), dtype=np.float32)
    if _trace:
        return full, res
    return full
